# revision 1
# baseline (speedup 1.0000x reference)
"""Causal multi-head attention on 8 Trainium2 NeuronCores.

Problem: x [4, 2048, 1024], 16 heads x dk=64, causal attention + output proj.

Sharding: 8 cores = 4 batches x 2 head-groups (8 heads each).
Each core computes, for its (batch b, head-group g):
    qT/kT = Wq_g x_b^T           [512, 2048]  ([head*dk, seq], f32r matmuls)
    v     = x_b Wv_g^T           [2048, 512]  ([seq, head*dk] natural layout)
    per head h, q-tile (512 wide), j-tile (128 wide, causal):
        sT = kT_h^T-block @ qT_h  -> [j 128, q 512] (row-tiled K=64 head pairs)
        p  = exp(sT/8) * causal_mask             (ACT exp -> f32r)
        PV[65, q] += v_ext[j,65]^T @ p           (col 64 of v_ext is ones -> Z)
        out_h[d, q] = PV[0:64] / PV[64]          (recip + PE K=1 broadcast)
    yT_partial = Wo_g^T-slice @ out_heads        [1024, 2048]
Host: y_b = (yT_{b,0} + yT_{b,1})^T.

All heavy matmuls use float32r (TF32-like, 1 cyc/row at free>=256, rel ~1e-4).
"""

import sys

if "/opt/trn_rl_repo" not in sys.path:
    sys.path.insert(0, "/opt/trn_rl_repo")

import numpy as np

import concourse.bass as bass
import concourse.mybir as mybir
from concourse import bacc, tile
from concourse.bass_utils import run_bass_kernel_spmd

P = 128
D_MODEL = 1024
NUM_HEADS = 16
DK = 64
B, S = 4, 2048
HG = NUM_HEADS // 2  # 8 heads per group
MG = HG * DK  # 512 columns per head-group
N_CORES = 8

QT = S // 512  # 4 q-tiles of 512
JT = S // P  # 16 j-tiles of 128
KT = D_MODEL // P  # 8 contraction tiles for projections
MSUB = MG // P  # 4 m-subtiles (head pairs)
NT = D_MODEL // P  # 8 output-proj n-tiles

F32 = mybir.dt.float32
F32R = mybir.dt.float32r
EXP = mybir.ActivationFunctionType.Exp

_CACHED_NC = None


def build_nc() -> bass.Bass:
    nc = bacc.Bacc("TRN2", target_bir_lowering=False, debug=False)

    xT = nc.dram_tensor("xT", [D_MODEL, S], F32R, kind="ExternalInput")
    wqT = nc.dram_tensor("wqT", [D_MODEL, MG], F32R, kind="ExternalInput")
    wkT = nc.dram_tensor("wkT", [D_MODEL, MG], F32R, kind="ExternalInput")
    wvT = nc.dram_tensor("wvT", [D_MODEL, MG], F32R, kind="ExternalInput")
    woT = nc.dram_tensor("woT", [MG, D_MODEL], F32R, kind="ExternalInput")
    masks = nc.dram_tensor("masks", [P, P], mybir.dt.bfloat16, kind="ExternalInput")
    yT = nc.dram_tensor("yT", [D_MODEL, S], F32, kind="ExternalOutput")

    xT_t = xT.rearrange("(kt p) s -> p kt s", p=P)
    wq_t = wqT.rearrange("(kt p) m -> p kt m", p=P)
    wk_t = wkT.rearrange("(kt p) m -> p kt m", p=P)
    wv_t = wvT.rearrange("(kt p) m -> p kt m", p=P)
    wo_t = woT.rearrange("(kt p) n -> p kt n", p=P)
    yT_t = yT.rearrange("(nt p) s -> p nt s", p=P)

    with tile.TileContext(nc) as tc:
        with (
            tc.tile_pool(name="wpool", bufs=1) as wpool,
            tc.tile_pool(name="qkv", bufs=1) as qkv,
            tc.tile_pool(name="oh", bufs=2) as ohp,
            tc.tile_pool(name="ys", bufs=4) as ysp,
        ):
            # ---- static tiles ----
            wo_sb = wpool.tile([P, MSUB, D_MODEL], F32R, tag="wo")
            for kt in range(MSUB):
                nc.sync.dma_start(wo_sb[:, kt], wo_t[:, kt])
            qT_sb = qkv.tile([P, MSUB, S], mybir.dt.bfloat16, tag="qT")
            kT_sb = qkv.tile([P, MSUB, S], mybir.dt.bfloat16, tag="kT")
            # v with ones column: [j-part, jt, head, dk+1]
            v_sb = qkv.tile([P, JT, HG, DK + 1], mybir.dt.bfloat16, tag="v")
            nc.vector.memset(v_sb[:, :, :, DK : DK + 1], 1.0)

            # ---- projections (weights + x stream live only in this phase) ----
            with (
                tc.tile_pool(name="wqkv", bufs=1) as wqkv,
                tc.tile_pool(name="xs", bufs=2) as xs,
                tc.tile_pool(name="ps_proj", bufs=4, space="PSUM") as ps_proj,
            ):
                # DMA order: x(st=0) first, then W kt-interleaved so the
                # kt=0 chunks (first matmul inputs) land early
                x_first = xs.tile([P, KT, 512], F32R, tag="x")
                w_sb = {}
                for name in ("q", "k", "v"):
                    w_sb[name] = wqkv.tile(
                        [P, KT, MG], F32R, tag=f"w{name}", name=f"w{name}"
                    )
                nc.sync.dma_start(x_first[:, 0], xT_t[:, 0, 0:512])
                nc.sync.dma_start(w_sb["q"][:, 0], wq_t[:, 0])
                for kt in range(1, KT):
                    nc.sync.dma_start(x_first[:, kt], xT_t[:, kt, 0:512])
                    nc.sync.dma_start(w_sb["q"][:, kt], wq_t[:, kt])
                for name, wsrc in (("k", wk_t), ("v", wv_t)):
                    for kt in range(KT):
                        nc.sync.dma_start(w_sb[name][:, kt], wsrc[:, kt])

                for st in range(QT):
                    ssl = slice(st * 512, (st + 1) * 512)
                    if st == 0:
                        x_t = x_first
                    else:
                        x_t = xs.tile([P, KT, 512], F32R, tag="x")
                        for kt in range(KT):
                            nc.sync.dma_start(x_t[:, kt], xT_t[:, kt, ssl])
                    # q, k: out[m 128, s 512] = w[:, kt, msl].T @ x[:, kt, ssl]
                    for name, dst in (("q", qT_sb), ("k", kT_sb)):
                        w = w_sb[name]
                        for mt in range(MSUB):
                            msl = slice(mt * P, (mt + 1) * P)
                            pt = ps_proj.tile([P, 512], F32, tag="pp")
                            for kt in range(KT):
                                nc.tensor.matmul(
                                    pt[:],
                                    w[:, kt, msl],
                                    x_t[:, kt],
                                    start=(kt == 0),
                                    stop=(kt == KT - 1),
                                )
                            nc.vector.tensor_copy(dst[:, mt, ssl], pt[:])
                    # v: out[s 128, m 512] = x[:, kt, ssub].T @ wv[:, kt, :]
                    for ssub in range(4):
                        jt = st * 4 + ssub
                        s0 = ssub * P
                        pt = ps_proj.tile([P, 512], F32, tag="pp")
                        for kt in range(KT):
                            nc.tensor.matmul(
                                pt[:],
                                x_t[:, kt, s0 : s0 + P],
                                w_sb["v"][:, kt],
                                start=(kt == 0),
                                stop=(kt == KT - 1),
                            )
                        nc.vector.tensor_copy(
                            v_sb[:, jt, :, 0:DK],
                            pt.rearrange("p (h d) -> p h d", h=HG),
                        )

            # ---- attention + output projection, per q-tile ----
            with (
                tc.tile_pool(name="attn", bufs=3) as attn,
                tc.tile_pool(name="attnc", bufs=1) as attnc,
                tc.tile_pool(name="ps_s", bufs=2, space="PSUM") as ps_s,
                tc.tile_pool(name="ps_o", bufs=1, space="PSUM") as ps_o,
                tc.tile_pool(name="ps_y", bufs=1, space="PSUM") as ps_y,
            ):
                # [128,2,128] window mask: valid iff (q - delta) >= j
                mask2 = attnc.tile([P, 2, P], mybir.dt.bfloat16, tag="mask2")
                nc.sync.dma_start(mask2[:, 0, :], masks[:])
                nc.sync.dma_start(mask2[:, 1, :], masks[:])
                # [1,128] selectors: sel_a lights partitions 0:64,
                # sel_b lights 64:128 (via two accumulating K=1 matmuls)
                sel_a = attnc.tile([1, P], F32R, tag="sel_a")
                sel_b = attnc.tile([1, P], F32R, tag="sel_b")
                nc.vector.memset(sel_a[:].bitcast(F32), 0.0)
                nc.vector.memset(sel_b[:].bitcast(F32), 0.0)
                nc.vector.memset(sel_a[0:1, 0:DK].bitcast(F32), 1.0)
                nc.vector.memset(sel_b[0:1, DK:P].bitcast(F32), 1.0)
                def emit_scores(qt, hp, jt):
                    """scores^T [j, q] for head pair hp, row-tiled K=64."""
                    jsl = slice(jt * P, (jt + 1) * P)
                    di = jt - qt * 4
                    delta = 128 * di if di >= 0 else 0
                    qsl_d = slice(qt * 512 + delta, (qt + 1) * 512)
                    ss = ps_s.tile([P, 2, 512], F32, tag="ss")
                    nc.tensor.matmul(
                        ss[:, 0, delta:],
                        kT_sb[0:DK, hp, jsl],
                        qT_sb[0:DK, hp, qsl_d],
                        start=True,
                        stop=True,
                    )
                    nc.tensor.matmul(
                        ss[:, 1, delta:],
                        kT_sb[DK:P, hp, jsl],
                        qT_sb[DK:P, hp, qsl_d],
                        start=True,
                        stop=True,
                    )
                    return ss, delta

                def emit_outproj(ohT_prev, qt_prev, nt):
                    qsl_p = slice(qt_prev * 512, (qt_prev + 1) * 512)
                    py = ps_y.tile([P, 512], F32, tag="py")
                    for mt in range(MSUB):
                        nc.tensor.matmul(
                            py[:],
                            wo_sb[:, mt, nt * P : (nt + 1) * P],
                            ohT_prev[:, mt, :],
                            start=(mt == 0),
                            stop=(mt == MSUB - 1),
                        )
                    y_sb = ysp.tile([P, 512], F32, tag="y")
                    if nt % 2 == 0:
                        nc.vector.tensor_copy(y_sb[:], py[:])
                    else:
                        nc.scalar.copy(y_sb[:], py[:])
                    nc.sync.dma_start(yT_t[:, nt, qsl_p], y_sb[:])

                prev = None  # (ohT, qt) with outproj still pending
                for qt in range(QT):
                    qsl = slice(qt * 512, (qt + 1) * 512)
                    njt = 4 * (qt + 1)
                    ohT = ohp.tile([P, MSUB, 512], F32R, tag="ohT")
                    work = [(hp, jt) for hp in range(MSUB) for jt in range(njt)]
                    drip = max(1, len(work) // NT)
                    pending_nt = 0
                    po = {}
                    pend = emit_scores(qt, 0, 0)
                    for wi, (hp, jt) in enumerate(work):
                        if prev is not None and wi % drip == drip - 1 and pending_nt < NT:
                            emit_outproj(prev[0], prev[1], pending_nt)
                            pending_nt += 1
                        if jt == 0:
                            po[hp] = (
                                ps_o.tile([DK + 1, 512], F32, tag="poA", name="po_a"),
                                ps_o.tile([DK + 1, 512], F32, tag="poB", name="po_b"),
                            )
                        ss, delta = pend
                        di = jt - qt * 4
                        pp = attn.tile([P, 2, 512], mybir.dt.bfloat16, tag="pp")
                        nc.scalar.activation(
                            pp[:, :, delta:], ss[:, :, delta:], EXP, scale=0.125
                        )
                        if di >= 0:  # mask the 128-wide staircase window
                            wsl = slice(delta, delta + P)
                            nc.vector.tensor_tensor(
                                pp[:, :, wsl],
                                pp[:, :, wsl],
                                mask2[:],
                                mybir.AluOpType.mult,
                            )
                        # issue next block's scores before this block's attn@V
                        # so the PE isn't blocked on the exp latency
                        if wi + 1 < len(work):
                            pend = emit_scores(qt, *work[wi + 1])
                        # attn @ V (col 64 of v_ext = ones -> Z in row 64)
                        po_a, po_b = po[hp]
                        nc.tensor.matmul(
                            po_a[:, delta:],
                            v_sb[:, jt, 2 * hp, :],
                            pp[:, 0, delta:],
                            start=(jt == 0),
                            stop=(jt == njt - 1),
                        )
                        nc.tensor.matmul(
                            po_b[:, delta:],
                            v_sb[:, jt, 2 * hp + 1, :],
                            pp[:, 1, delta:],
                            start=(jt == 0),
                            stop=(jt == njt - 1),
                        )
                        if jt == njt - 1:
                            # drain PV + Z (releases po banks), broadcast
                            # Z via two accumulating selector matmuls into a
                            # py-tag psum (not po -- keeps po turnover fast),
                            # then ONE [128,512] reciprocal + multiply
                            z2 = attnc.tile([1, 1024], F32R, tag="z2")
                            nc.vector.tensor_copy(z2[0:1, 0:512], po_a[DK : DK + 1, :])
                            nc.vector.tensor_copy(z2[0:1, 512:], po_b[DK : DK + 1, :])
                            dst = ohT[:, hp, :]
                            nc.scalar.copy(dst[0:DK], po_a[0:DK, :])
                            nc.vector.tensor_copy(dst[DK:P], po_b[0:DK, :])
                            bcz = ps_y.tile([P, 512], F32, tag="bcz", name="bcz")
                            nc.tensor.matmul(
                                bcz[:], sel_a[:], z2[0:1, 0:512],
                                start=True, stop=False,
                            )
                            nc.tensor.matmul(
                                bcz[:], sel_b[:], z2[0:1, 512:],
                                start=False, stop=True,
                            )
                            bcr = attnc.tile([P, 512], F32, tag="bcr")
                            with nc.allow_low_precision(reason="softmax recip"):
                                nc.vector.reciprocal(bcr[:], bcz[:])
                            nc.vector.tensor_tensor(
                                dst, dst, bcr[:], mybir.AluOpType.mult
                            )
                    # flush any outproj of the previous qt not yet dripped
                    if prev is not None:
                        while pending_nt < NT:
                            emit_outproj(prev[0], prev[1], pending_nt)
                            pending_nt += 1
                    prev = (ohT, qt)
                # final qt's output projection
                for nt in range(NT):
                    emit_outproj(prev[0], prev[1], nt)

    nc.finalize()
    return nc


def _get_nc() -> bass.Bass:
    global _CACHED_NC
    if _CACHED_NC is None:
        _CACHED_NC = build_nc()
    return _CACHED_NC


def _make_masks() -> np.ndarray:
    j = np.arange(P)[:, None]
    w = np.arange(P)[None, :]
    import ml_dtypes
    return (w >= j).astype(ml_dtypes.bfloat16)


def kernel(x, q_heads, k_heads, v_heads, output_proj):
    x = np.asarray(x, dtype=np.float32)
    q_heads = np.asarray(q_heads, dtype=np.float32)
    k_heads = np.asarray(k_heads, dtype=np.float32)
    v_heads = np.asarray(v_heads, dtype=np.float32)
    output_proj = np.asarray(output_proj, dtype=np.float32)

    masks = _make_masks()
    in_maps = []
    for core in range(N_CORES):
        b, g = divmod(core, 2)
        gsl = slice(g * MG, (g + 1) * MG)
        in_maps.append(
            {
                "xT": np.ascontiguousarray(x[b].T),
                "wqT": np.ascontiguousarray(q_heads[gsl].T),
                "wkT": np.ascontiguousarray(k_heads[gsl].T),
                "wvT": np.ascontiguousarray(v_heads[gsl].T),
                "woT": np.ascontiguousarray(output_proj[:, gsl].T),
                "masks": masks,
            }
        )

    nc = _get_nc()
    res = run_bass_kernel_spmd(nc, in_maps, list(range(N_CORES)))
    y = np.empty((B, S, D_MODEL), np.float32)
    for b in range(B):
        acc = res.results[2 * b]["yT"] + res.results[2 * b + 1]["yT"]
        y[b] = acc.T
    return y



# revision 5
# speedup vs baseline: 14329.6039x; 14329.6039x over previous
"""Causal multi-head attention on 8 Trainium2 NeuronCores.

Problem: x [4, 2048, 1024], 16 heads x dk=64, causal attention + output proj.

Sharding: 8 cores = 4 batches x 2 head-groups (8 heads each).
Each core computes, for its (batch b, head-group g), all in bf16:
    qT/kT = Wq_g x_b^T           [512, 2048]  ([head*dk, seq])
    v     = x_b Wv_g^T           [2048, 512]  ([seq, head*dk])
    per head-pair hp (q-tile 512 wide, j-tile 128 wide, causal):
        sT pair = kT_h^T-block @ qT_h   [j 128, q 512]  (row-packed K=64 pair)
        pp  = exp(sT/8) * causal_mask   (ACT exp -> bf16)
        po[0:64]  += v_a^T @ pp_a  \  col-packed concurrent pair
        po[64:128]+= v_b^T @ pp_b  /  (tile_position (0,0) / (0,64))
        S_acc += pp                     (DVE, bf16 -> softmax denominator)
      strip end: Z = ones^T @ S_acc (2 concurrent MMs), recip on DVE,
        broadcast 1/Z via K=1 selector matmuls, ohT = po * bcr (DVE).
    yT_partial = Wo_g^T-slice @ ohT    [1024, 2048]
Host: y_b = (yT_{b,0} + yT_{b,1})^T.

Schedule: st=0 projections up front; projections for st=qt+1 and the
output projections of earlier q-tiles are dripped between attention
blocks so the PE never idles and the ACT-engine exp is hidden.
"""

import sys

if "/opt/trn_rl_repo" not in sys.path:
    sys.path.insert(0, "/opt/trn_rl_repo")

import numpy as np

import concourse.bass as bass
import concourse.mybir as mybir
from concourse import bacc, tile
from concourse.bass_utils import run_bass_kernel_spmd

P = 128
D_MODEL = 1024
NUM_HEADS = 16
DK = 64
B, S = 4, 2048
HG = NUM_HEADS // 2  # 8 heads per group
MG = HG * DK  # 512 columns per head-group
N_CORES = 8

QT = S // 512  # 4 q-tiles of 512
JT = S // P  # 16 j-tiles of 128
KT = D_MODEL // P  # 8 contraction tiles for projections
MSUB = MG // P  # 4 m-subtiles (head pairs)
NT = D_MODEL // P  # 8 output-proj n-tiles

F32 = mybir.dt.float32
F32R = mybir.dt.float32r
BF16 = mybir.dt.bfloat16
EXP = mybir.ActivationFunctionType.Exp
MULT = mybir.AluOpType.mult
ADD = mybir.AluOpType.add

_CACHED_NC = None


def build_nc() -> bass.Bass:
    nc = bacc.Bacc("TRN2", target_bir_lowering=False, debug=False)

    xT = nc.dram_tensor("xT", [D_MODEL, S], BF16, kind="ExternalInput")
    wqT = nc.dram_tensor("wqT", [D_MODEL, MG], BF16, kind="ExternalInput")
    wkT = nc.dram_tensor("wkT", [D_MODEL, MG], BF16, kind="ExternalInput")
    wvT = nc.dram_tensor("wvT", [D_MODEL, MG], BF16, kind="ExternalInput")
    woT = nc.dram_tensor("woT", [MG, D_MODEL], BF16, kind="ExternalInput")
    masks = nc.dram_tensor("masks", [P, P], BF16, kind="ExternalInput")
    yT = nc.dram_tensor("yT", [D_MODEL, S], F32, kind="ExternalOutput")

    xT_t = xT.rearrange("(kt p) s -> p kt s", p=P)
    wq_t = wqT.rearrange("(kt p) m -> p kt m", p=P)
    wk_t = wkT.rearrange("(kt p) m -> p kt m", p=P)
    wv_t = wvT.rearrange("(kt p) m -> p kt m", p=P)
    wo_t = woT.rearrange("(kt p) n -> p kt n", p=P)
    yT_t = yT.rearrange("(nt p) s -> p nt s", p=P)

    with tile.TileContext(nc) as tc:
        with (
            tc.tile_pool(name="wpool", bufs=1) as wpool,
            tc.tile_pool(name="qkv", bufs=1) as qkv,
            tc.tile_pool(name="wqkv", bufs=1) as wqkv,
            tc.tile_pool(name="xs", bufs=2) as xs,
            tc.tile_pool(name="sacc", bufs=2) as saccp,
            tc.tile_pool(name="oh", bufs=4) as ohp,
            tc.tile_pool(name="attn", bufs=3) as attn,
            tc.tile_pool(name="attnc", bufs=1) as attnc,
            tc.tile_pool(name="ys", bufs=4) as ysp,
            tc.tile_pool(name="ps_s", bufs=2, space="PSUM") as ps_s,
            tc.tile_pool(name="ps_o", bufs=1, space="PSUM") as ps_o,
            tc.tile_pool(name="ps_m", bufs=2, space="PSUM") as ps_m,
        ):
            # ---- static tiles ----
            wo_sb = wpool.tile([P, MSUB, D_MODEL], BF16, tag="wo")
            qT_sb = qkv.tile([P, MSUB, S], BF16, tag="qT")
            kT_sb = qkv.tile([P, MSUB, S], BF16, tag="kT")
            v_sb = qkv.tile([P, JT, HG, DK], BF16, tag="v")
            mask2 = attnc.tile([P, 2, P], BF16, tag="mask2")
            # [1,128] selectors: sel_a lights partitions 0:64, sel_b 64:128
            sel_a = attnc.tile([1, P], BF16, tag="sel_a")
            sel_b = attnc.tile([1, P], BF16, tag="sel_b")
            ones_sb = attnc.tile([P, 1], BF16, tag="ones")
            zr = attnc.tile([1, 2, 512], BF16, tag="zr")
            bcr_sb = attnc.tile([P, 512], BF16, tag="bcr_sb")
            nc.vector.memset(sel_a[:], 0.0)
            nc.vector.memset(sel_b[:], 0.0)
            nc.vector.memset(sel_a[0:1, 0:DK], 1.0)
            nc.vector.memset(sel_b[0:1, DK:P], 1.0)
            nc.vector.memset(ones_sb[:], 1.0)
            nc.sync.dma_start(mask2[:, 0, :], masks[:])
            nc.sync.dma_start(mask2[:, 1, :], masks[:])

            # ---- input DMAs (x0+wq interleaved so first matmuls start early)
            w_sb = {}
            for name in ("q", "k", "v"):
                w_sb[name] = wqkv.tile([P, KT, MG], BF16, tag=f"w{name}", name=f"w{name}")
            x_tiles = {}
            for st in (0, 1):
                x_tiles[st] = xs.tile([P, KT, 512], BF16, tag="x", name=f"x{st}")
            for kt in range(KT):
                nc.sync.dma_start(x_tiles[0][:, kt], xT_t[:, kt, 0:512])
                nc.sync.dma_start(w_sb["q"][:, kt], wq_t[:, kt])
            for name, wsrc in (("k", wk_t), ("v", wv_t)):
                for kt in range(KT):
                    nc.sync.dma_start(w_sb[name][:, kt], wsrc[:, kt])
            for kt in range(MSUB):
                nc.sync.dma_start(wo_sb[:, kt], wo_t[:, kt])
            for kt in range(KT):
                nc.sync.dma_start(x_tiles[1][:, kt], xT_t[:, kt, 512:1024])

            # ---- projection group emitters ----
            def emit_proj_qk(name, dst, mt, st):
                x_t = x_tiles[st]
                ssl = slice(st * 512, (st + 1) * 512)
                w = w_sb[name]
                msl = slice(mt * P, (mt + 1) * P)
                pt = ps_m.tile([P, 512], F32, tag="ms", name=f"p{name}{st}{mt}")
                for kt in range(KT):
                    nc.tensor.matmul(
                        pt[:], w[:, kt, msl], x_t[:, kt],
                        start=(kt == 0), stop=(kt == KT - 1),
                    )
                nc.vector.tensor_copy(dst[:, mt, ssl], pt[:])

            def emit_proj_v(ssub, st):
                x_t = x_tiles[st]
                jt_ = st * 4 + ssub
                s0 = ssub * P
                pt = ps_m.tile([P, 512], F32, tag="ms", name=f"pv{st}{ssub}")
                for kt in range(KT):
                    nc.tensor.matmul(
                        pt[:], x_t[:, kt, s0 : s0 + P], w_sb["v"][:, kt],
                        start=(kt == 0), stop=(kt == KT - 1),
                    )
                nc.vector.tensor_copy(
                    v_sb[:, jt_, :, :], pt.rearrange("p (h d) -> p h d", h=HG)
                )

            def proj_items(st):
                items = []
                for name, dst in (("q", qT_sb), ("k", kT_sb)):
                    for mt in range(MSUB):
                        items.append(lambda n=name, d=dst, m=mt: emit_proj_qk(n, d, m, st))
                for ssub in range(4):
                    items.append(lambda s=ssub: emit_proj_v(s, st))
                return items

            def emit_outproj(ohT_prev, qt_prev, nt):
                qsl_p = slice(qt_prev * 512, (qt_prev + 1) * 512)
                py = ps_m.tile([P, 512], F32, tag="ms", name=f"py{qt_prev}{nt}")
                for mt in range(MSUB):
                    nc.tensor.matmul(
                        py[:],
                        wo_sb[:, mt, nt * P : (nt + 1) * P],
                        ohT_prev[:, mt, :],
                        start=(mt == 0),
                        stop=(mt == MSUB - 1),
                    )
                y_sb = ysp.tile([P, 512], F32, tag="y")
                nc.vector.tensor_copy(y_sb[:], py[:])
                nc.sync.dma_start(yT_t[:, nt, qsl_p], y_sb[:])

            def outproj_items(ohT_prev, qt_prev):
                return [
                    lambda n=nt: emit_outproj(ohT_prev, qt_prev, n)
                    for nt in range(NT)
                ]

            def emit_scores(qt, hp, jt):
                """scores^T [j, q] pair for head pair hp, row-packed K=64."""
                jsl = slice(jt * P, (jt + 1) * P)
                di = jt - qt * 4
                delta = 128 * di if di >= 0 else 0
                qsl_d = slice(qt * 512 + delta, (qt + 1) * 512)
                ss = ps_s.tile([P, 2, 512], F32, tag="ss")
                nc.tensor.matmul(
                    ss[:, 0, delta:],
                    kT_sb[0:DK, hp, jsl],
                    qT_sb[0:DK, hp, qsl_d],
                    start=True, stop=True,
                )
                nc.tensor.matmul(
                    ss[:, 1, delta:],
                    kT_sb[DK:P, hp, jsl],
                    qT_sb[DK:P, hp, qsl_d],
                    start=True, stop=True,
                )
                return ss, delta

            # ---- phase A: st=0 projections ----
            for it in proj_items(0):
                it()

            # ---- main loop: attention with dripped proj/outproj ----
            oh_tiles = {}
            for qt in range(QT):
                njt = 4 * (qt + 1)
                work = [(hp, jt) for hp in range(MSUB) for jt in range(njt)]
                ohT = ohp.tile([P, MSUB, 512], BF16, tag="ohT")
                oh_tiles[qt] = ohT

                # x prefetch for the tile dripped NEXT qt
                if qt + 2 <= 3 and (qt + 2) not in x_tiles:
                    x_tiles[qt + 2] = xs.tile([P, KT, 512], BF16, tag="x", name=f"x{qt+2}")
                    ssl = slice((qt + 2) * 512, (qt + 3) * 512)
                    for kt in range(KT):
                        nc.sync.dma_start(x_tiles[qt + 2][:, kt], xT_t[:, kt, ssl])

                # drip: proj for st=qt+1; outproj reserved for the
                # ACT-heavy late q-tiles (qt2: y(q0); qt3: y(q1), y(q2))
                drip = []
                if qt < 3:
                    drip += proj_items(qt + 1)
                if qt == 2:
                    drip += outproj_items(oh_tiles[0], 0)
                if qt == 3:
                    drip += outproj_items(oh_tiles[1], 1)
                    drip += outproj_items(oh_tiles[2], 2)

                per = len(drip) / len(work)
                dripped = 0
                sacc_t = None
                po_a = po_b = None
                pend = emit_scores(qt, 0, 0)
                for wi, (hp, jt) in enumerate(work):
                    if jt == 0:
                        po_a = ps_o.tile([P, 512], F32, tag="poA", name="po_a")
                        po_b = ps_o.tile([P, 512], F32, tag="poB", name="po_b")
                    ss, delta = pend
                    di = jt - qt * 4
                    pp = attn.tile([P, 2, 512], BF16, tag="pp")
                    nc.scalar.activation(
                        pp[:, :, delta:], ss[:, :, delta:], EXP, scale=0.125
                    )
                    if di >= 0:  # mask the 128-wide staircase window
                        wsl = slice(delta, delta + P)
                        nc.vector.tensor_tensor(
                            pp[:, :, wsl], pp[:, :, wsl], mask2[:], MULT
                        )
                    # softmax denominator partial sums (per j-slot)
                    if jt == 0:
                        sacc_t = saccp.tile([P, 2, 512], BF16, tag="sa")
                        nc.vector.tensor_copy(sacc_t[:], pp[:])
                    else:
                        nc.vector.tensor_tensor(
                            sacc_t[:, :, delta:],
                            sacc_t[:, :, delta:],
                            pp[:, :, delta:],
                            ADD,
                        )
                    # drip PE-filler work (projections / output proj)
                    while dripped < int(per * (wi + 1)) and dripped < len(drip):
                        drip[dripped]()
                        dripped += 1
                    # issue next block's scores before this block's attn@V
                    if wi + 1 < len(work):
                        pend = emit_scores(qt, *work[wi + 1])
                    # attn @ V: col-packed concurrent pair (0,0)/(0,64)
                    nc.tensor.matmul(
                        po_a[0:DK, delta:],
                        v_sb[:, jt, 2 * hp, :],
                        pp[:, 0, delta:],
                        start=(jt == 0),
                        stop=(jt == njt - 1),
                    )
                    nc.tensor.matmul(
                        po_b[DK:P, delta:],
                        v_sb[:, jt, 2 * hp + 1, :],
                        pp[:, 1, delta:],
                        start=(jt == 0),
                        stop=(jt == njt - 1),
                    )
                    if jt == njt - 1:
                        # softmax denominators: two concurrent K=128 M=1
                        # matmuls (col groups 0 / 32, separate psum banks)
                        z2a = ps_m.tile([P, 512], F32, tag="ms", name="z2a")
                        z2b = ps_m.tile([P, 512], F32, tag="ms", name="z2b")
                        nc.tensor.matmul(
                            z2a[0:1, :], ones_sb[:], sacc_t[:, 0, :],
                            start=True, stop=True,
                        )
                        nc.tensor.matmul(
                            z2b[32:33, :], ones_sb[:], sacc_t[:, 1, :],
                            start=True, stop=True,
                        )
                        with nc.allow_low_precision(reason="softmax recip"):
                            nc.vector.reciprocal(zr[0:1, 0, :], z2a[0:1, :])
                            nc.vector.reciprocal(zr[0:1, 1, :], z2b[32:33, :])
                        # broadcast 1/Z to all partitions via selector MMs
                        bcr = ps_m.tile([P, 512], F32, tag="ms", name="bcr")
                        nc.tensor.matmul(
                            bcr[:], sel_a[:], zr[0:1, 0, :],
                            start=True, stop=False,
                        )
                        nc.tensor.matmul(
                            bcr[:], sel_b[:], zr[0:1, 1, :],
                            start=False, stop=True,
                        )
                        # bounce 1/Z to SBUF (ACT engine), then normalize
                        nc.scalar.copy(bcr_sb[:], bcr[:])
                        nc.vector.tensor_tensor(
                            ohT[0:DK, hp, :], po_a[0:DK, :], bcr_sb[0:DK, :], MULT
                        )
                        nc.vector.tensor_tensor(
                            ohT[DK:P, hp, :], po_b[DK:P, :], bcr_sb[DK:P, :], MULT
                        )
                while dripped < len(drip):
                    drip[dripped]()
                    dripped += 1
            # final q-tile's output projection
            for it in outproj_items(oh_tiles[3], 3):
                it()

    nc.finalize()
    return nc


def _get_nc() -> bass.Bass:
    global _CACHED_NC
    if _CACHED_NC is None:
        _CACHED_NC = build_nc()
    return _CACHED_NC


def _make_masks() -> np.ndarray:
    j = np.arange(P)[:, None]
    w = np.arange(P)[None, :]
    import ml_dtypes

    return (w >= j).astype(ml_dtypes.bfloat16)


def make_in_maps(inputs):
    import ml_dtypes

    bf16 = ml_dtypes.bfloat16
    x = np.asarray(inputs["x"], dtype=np.float32)
    q_heads = np.asarray(inputs["q_heads"], dtype=np.float32)
    k_heads = np.asarray(inputs["k_heads"], dtype=np.float32)
    v_heads = np.asarray(inputs["v_heads"], dtype=np.float32)
    output_proj = np.asarray(inputs["output_proj"], dtype=np.float32)

    masks = _make_masks()
    in_maps = []
    for core in range(N_CORES):
        b, g = divmod(core, 2)
        gsl = slice(g * MG, (g + 1) * MG)
        in_maps.append(
            {
                "xT": np.ascontiguousarray(x[b].T).astype(bf16),
                "wqT": np.ascontiguousarray(q_heads[gsl].T).astype(bf16),
                "wkT": np.ascontiguousarray(k_heads[gsl].T).astype(bf16),
                "wvT": np.ascontiguousarray(v_heads[gsl].T).astype(bf16),
                "woT": np.ascontiguousarray(output_proj[:, gsl].T).astype(bf16),
                "masks": masks,
            }
        )
    return in_maps


def kernel(x, q_heads, k_heads, v_heads, output_proj):
    in_maps = make_in_maps(
        {
            "x": x,
            "q_heads": q_heads,
            "k_heads": k_heads,
            "v_heads": v_heads,
            "output_proj": output_proj,
        }
    )
    nc = _get_nc()
    res = run_bass_kernel_spmd(nc, in_maps, list(range(N_CORES)))
    y = np.empty((B, S, D_MODEL), np.float32)
    for b in range(B):
        acc = res.results[2 * b]["yT"] + res.results[2 * b + 1]["yT"]
        y[b] = acc.T
    return y
